# revision 1
# baseline (speedup 1.0000x reference)
"""GCN encoder (5-layer GCNConv + global mean pool) on 8 Trainium2 NeuronCores.

Strategy (node sharding):
  - 10000 nodes split contiguously across 8 cores (1250/core, padded to 1280).
  - Each layer: per-core GEMM (h @ W, fp16 operands, fp32 PSUM) ->
    AllGather of the fp16 hW slices into pair-shared DRAM ->
    dma_gather of per-edge source rows (dst-sorted, chunked 128 edges) ->
    segment-sum as one-hot matmul per chunk (Seg[128e,128d].T @ msgs[128e,fo]
    accumulated in PSUM per 128-dst tile; GCN norm folded into Seg values) ->
    bias + relu (DVE) -> PE transpose to keep h^T for the next GEMM.
  - Mean-pool as matmul with 1/count one-hot, AllReduce over cores.

The graph structure (edge sort, one-hot Seg with norm, gather indices,
pool matrix) is preprocessed on host; all FLOPs on x/W run on device.
"""
import sys

import numpy as np

sys.path.insert(0, "/opt/trn_rl_repo")

import concourse.bacc as bacc
import concourse.bass as bass  # noqa: F401
import concourse.mybir as mybir
import concourse.tile as tile
from concourse import bass_utils

dt = mybir.dt

N = 10000
E = 150000
G = 64
DIN = 128
DHID = 512
DOUT = 128
C = 8
NPC = N // C          # 1250 nodes per core
NTILE = 10            # ceil(1250/128)
NPAD = NTILE * 128    # 1280 padded rows per core
NROWS = C * NPAD      # 10240 rows in the allgathered table
FO = [DHID, DHID, DHID, DHID, DOUT]
FIT = [1, 4, 4, 4, 4]  # fi tiles per layer (fi=128 for L1, 512 for L2-5)


def _preprocess(edge_index, batch):
    """Build per-core gather indices, one-hot Seg blocks, and pool matrix."""
    src = np.concatenate([edge_index[0], np.arange(N, dtype=np.int64)])
    dst = np.concatenate([edge_index[1], np.arange(N, dtype=np.int64)])
    deg = np.bincount(dst, minlength=N).astype(np.float64)
    dinv = np.where(deg > 0, 1.0 / np.sqrt(deg), 0.0)
    norm = (dinv[src] * dinv[dst]).astype(np.float32)

    core = dst // NPC
    t_of = (dst % NPC) // 128
    dloc = (dst % NPC) % 128

    # edge counts per (core, tile) -> uniform chunk budget T_pad
    cnt = np.zeros((C, NTILE), np.int64)
    np.add.at(cnt, (core, t_of), 1)
    t_pad = int(np.ceil(cnt.max() / 128))
    nchunk = NTILE * t_pad

    # slot of each edge inside its (core, tile) bucket
    order = np.lexsort((dst, t_of, core))
    s_src, s_core, s_t, s_dloc, s_norm = (
        src[order], core[order], t_of[order], dloc[order], norm[order])
    # position within bucket
    bucket = s_core * NTILE + s_t
    start = np.zeros(C * NTILE, np.int64)
    start[1:] = np.cumsum(np.bincount(bucket, minlength=C * NTILE))[:-1]
    pos = np.arange(len(s_src)) - start[bucket]

    chunk = s_t * t_pad + pos // 128     # chunk id within core
    erow = pos % 128                     # row within chunk

    # padded-row index of each source node in the allgathered table
    srow = (s_src // NPC) * NPAD + (s_src % NPC)

    gidx = np.zeros((C, nchunk * 128), np.int16)
    seg = np.zeros((C, 128, nchunk, 128), np.float16)
    gidx[s_core, chunk * 128 + erow] = srow.astype(np.int16)
    seg[s_core, erow, chunk, s_dloc] = s_norm.astype(np.float16)

    # idx wrap: logical idx i -> partition i%16, column i//16; replicate x8
    gidx_w = np.ascontiguousarray(
        np.tile(gidx.reshape(C, -1, 16).transpose(0, 2, 1), (1, 8, 1)))

    # pool matrix [C, 128, NTILE, G]: 1/count at (node row, graph)
    gcnt = np.bincount(batch, minlength=G).astype(np.float64)
    inv = (1.0 / np.maximum(gcnt, 1.0))
    pool = np.zeros((C, 128, NTILE, G), np.float16)
    nodes = np.arange(N)
    pc, pr = nodes // NPC, nodes % NPC
    pool[pc, pr % 128, pr // 128, batch] = inv[batch].astype(np.float16)

    return gidx_w, seg, pool, t_pad, nchunk


def _build(t_pad, nchunk):
    nc = bacc.Bacc("TRN2", target_bir_lowering=False, debug=False, num_devices=C)

    xs_in = nc.dram_tensor("xs_in", [NPC, DIN], dt.float32, kind="ExternalInput")
    w_in = [nc.dram_tensor(f"w{i}_in", [DIN if i == 0 else DHID, FO[i]],
                           dt.float32, kind="ExternalInput") for i in range(5)]
    b_in = [nc.dram_tensor(f"b{i}_in", [128, FO[i]], dt.float32,
                           kind="ExternalInput") for i in range(5)]
    seg_in = nc.dram_tensor("seg_in", [128, nchunk, 128], dt.float16,
                            kind="ExternalInput")
    gidx_in = nc.dram_tensor("gidx_in", [128, nchunk * 8], dt.int16,
                             kind="ExternalInput")
    pool_in = nc.dram_tensor("pool_in", [128, NTILE, G], dt.float16,
                             kind="ExternalInput")
    id_in = nc.dram_tensor("id_in", [128, 128], dt.float16, kind="ExternalInput")
    out = nc.dram_tensor("out", [G, DOUT], dt.float32, kind="ExternalOutput")

    hw_sh = nc.dram_tensor("hw_sh", [NROWS, DHID], dt.float16, addr_space="Shared")
    hw_sh5 = nc.dram_tensor("hw_sh5", [NROWS, DOUT], dt.float16, addr_space="Shared")
    pool_sh = nc.dram_tensor("pool_sh", [G, DOUT], dt.float32, addr_space="Shared")
    bounce = nc.dram_tensor("bounce", [NPAD, DHID], dt.float16)
    bounce5 = nc.dram_tensor("bounce5", [NPAD, DOUT], dt.float16)
    pool_bounce = nc.dram_tensor("pool_bounce", [G, DOUT], dt.float32)

    tsz = [128] * (NTILE - 1) + [NPC - 128 * (NTILE - 1)]

    with tile.TileContext(nc) as tc:
        with (
            tc.tile_pool(name="const", bufs=1) as cp,
            tc.tile_pool(name="work", bufs=2) as wp,
            tc.tile_pool(name="msgp", bufs=3) as mp,
            tc.tile_pool(name="gemm_ps", bufs=2, space="PSUM") as gps,
            tc.tile_pool(name="agg_ps", bufs=2, space="PSUM") as aps,
            tc.tile_pool(name="tp_ps", bufs=2, space="PSUM") as tps,
            tc.tile_pool(name="pool_ps", bufs=1, space="PSUM") as pps,
        ):
            # ---- resident tensors ----
            seg_sb = cp.tile([128, nchunk, 128], dt.float16)
            nc.sync.dma_start(out=seg_sb[:, :, :], in_=seg_in[:, :, :])
            gidx_sb = cp.tile([128, nchunk * 8], dt.int16)
            nc.sync.dma_start(out=gidx_sb[:, :], in_=gidx_in[:, :])
            pool_sb = cp.tile([128, NTILE, G], dt.float16)
            nc.sync.dma_start(out=pool_sb[:, :, :], in_=pool_in[:, :, :])
            id16 = cp.tile([128, 128], dt.float16)
            nc.sync.dma_start(out=id16[:, :], in_=id_in[:, :])
            breps = cp.tile([128, 4, DHID], dt.float32)
            for l in range(4):
                nc.sync.dma_start(out=breps[:, l, :], in_=b_in[l][:, :])
            brep5 = cp.tile([128, DOUT], dt.float32)
            nc.sync.dma_start(out=brep5[:, :], in_=b_in[4][:, :])

            # weights -> fp16 tiles. slots: L1 -> w16[:,0,:]; L2..L4 -> 1+4(l-1)+j
            w16 = cp.tile([128, 13, DHID], dt.float16)
            w516 = cp.tile([128, 4, DOUT], dt.float16)
            for l in range(5):
                for j in range(FIT[l]):
                    wstage = wp.tile([128, FO[l]], dt.float32, tag="wstage")
                    nc.sync.dma_start(
                        out=wstage[:, :], in_=w_in[l][j * 128:(j + 1) * 128, :])
                    if l < 4:
                        nc.vector.tensor_copy(w16[:, (0 if l == 0 else 1 + 4 * (l - 1)) + j, :], wstage[:, :])
                    else:
                        nc.vector.tensor_copy(w516[:, j, :], wstage[:, :])

            # hT: transposed activations [128fi, tile, fi_tile, 128n]
            hT = cp.tile([128, NTILE, 4, 128], dt.float16)
            h_out = cp.tile([128, NTILE, DOUT], dt.float16)

            # x slice -> hT[:, t, 0, :]
            for t in range(NTILE):
                xstage = wp.tile([128, 128], dt.float16, tag="xstage")
                xraw = wp.tile([128, 128], dt.float32, tag="xraw")
                if tsz[t] < 128:
                    nc.vector.memset(xraw[:, :], 0.0)
                nc.sync.dma_start(
                    out=xraw[:tsz[t], :],
                    in_=xs_in[t * 128: t * 128 + tsz[t], :])
                nc.vector.tensor_copy(xstage[:, :], xraw[:, :])
                pt0 = tps.tile([128, DHID], dt.float16, tag="pt")
                nc.tensor.transpose(pt0[:, :128], xstage[:, :], id16[:, :])
                nc.vector.tensor_copy(hT[:, t, 0, :], pt0[:, :128])

            # ---- layers ----
            for l in range(5):
                fo = FO[l]
                fit = FIT[l]
                gsh = hw_sh if l < 4 else hw_sh5
                gbounce = bounce if l < 4 else bounce5

                # GEMM h @ W -> hw16 staging (node-major fp16)
                hw16 = wp.tile([128, NTILE, fo], dt.float16, tag="hw16")
                for t in range(NTILE):
                    pg = gps.tile([128, fo], dt.float32, tag="pg")
                    for j in range(fit):
                        wslot = (w16[:, (0 if l == 0 else 1 + 4 * (l - 1)) + j, :fo]
                                 if l < 4 else w516[:, j, :])
                        nc.tensor.matmul(
                            pg[:, :], hT[:, t, j, :], wslot,
                            start=(j == 0), stop=(j == fit - 1))
                    nc.vector.tensor_copy(hw16[:, t, :], pg[:, :])

                nc.sync.dma_start(
                    out=gbounce.ap().rearrange("(t p) f -> p t f", p=128),
                    in_=hw16[:, :, :])
                nc.gpsimd.collective_compute(
                    "AllGather", mybir.AluOpType.bypass,
                    replica_groups=[list(range(C))],
                    ins=[gbounce.ap().opt()],
                    outs=[gsh.ap().opt()])

                # aggregate per dst tile; gathers split into ring-safe batches
                safe = (t_pad + 1) // 2
                for t in range(NTILE):
                    pa = aps.tile([128, fo], dt.float32, tag="pa")
                    for b in range((t_pad + safe - 1) // safe):
                        k0 = b * safe
                        k1 = min(t_pad, k0 + safe)
                        msgs = mp.tile([128, safe, fo], dt.float16, tag="msgs")
                        nc.gpsimd.dma_gather(
                            out_ap=msgs[:, :k1 - k0, :],
                            in_ap=gsh[:, :],
                            idxs_ap=gidx_sb[:, (t * t_pad + k0) * 8:
                                            (t * t_pad + k1) * 8],
                            num_idxs=(k1 - k0) * 128,
                            num_idxs_reg=(k1 - k0) * 128,
                            elem_size=fo,
                            single_packet=False)
                        for k in range(k0, k1):
                            nc.tensor.matmul(
                                pa[:, :], seg_sb[:, t * t_pad + k, :],
                                msgs[:, k - k0, :],
                                start=(k == 0), stop=(k == t_pad - 1))
                    hsum = wp.tile([128, fo], dt.float32, tag="hsum")
                    nc.vector.tensor_tensor(
                        hsum[:, :], pa[:, :],
                        breps[:, l, :fo] if l < 4 else brep5[:, :],
                        mybir.AluOpType.add)
                    if l < 4:
                        hnm = wp.tile([128, fo], dt.float16, tag="hnm")
                        nc.vector.tensor_scalar_max(hnm[:, :], hsum[:, :], 0.0)
                        pt = tps.tile([128, fo], dt.float16, tag="pt")
                        for j in range(fo // 128):
                            nc.tensor.transpose(
                                pt[:, j * 128:(j + 1) * 128],
                                hnm[:, j * 128:(j + 1) * 128], id16[:, :])
                        nc.vector.tensor_copy(
                            hT[:, t, :, :].rearrange("p a b -> p (a b)"),
                            pt[:, :])
                    else:
                        nc.vector.tensor_scalar_max(
                            h_out[:, t, :], hsum[:, :], 0.0)

            # ---- mean pool ----
            pp = pps.tile([64, DOUT], dt.float32)
            for t in range(NTILE):
                nc.tensor.matmul(
                    pp[:, :], pool_sb[:, t, :64], h_out[:, t, :],
                    start=(t == 0), stop=(t == NTILE - 1))
            pres = wp.tile([64, DOUT], dt.float32, tag="pres")
            nc.vector.tensor_copy(pres[:, :], pp[:, :])
            nc.sync.dma_start(out=pool_bounce[:, :], in_=pres[:, :])
            nc.gpsimd.collective_compute(
                "AllReduce", mybir.AluOpType.add,
                replica_groups=[list(range(C))],
                ins=[pool_bounce.ap().opt()],
                outs=[pool_sh.ap().opt()])
            ores = wp.tile([64, DOUT], dt.float32, tag="ores")
            nc.sync.dma_start(out=ores[:, :], in_=pool_sh[:, :])
            nc.sync.dma_start(out=out[:, :], in_=ores[:, :])

    nc.compile()
    return nc


_CACHE = {}


def _get_program(t_pad, nchunk):
    key = (t_pad, nchunk)
    if key not in _CACHE:
        _CACHE[key] = _build(t_pad, nchunk)
    return _CACHE[key]


def make_in_maps(inputs):
    edge_index = np.asarray(inputs["edge_index"])
    batch = np.asarray(inputs["batch"])
    x = np.asarray(inputs["x"], dtype=np.float32)
    gidx_w, seg, pool, t_pad, nchunk = _preprocess(edge_index, batch)
    ident = np.eye(128, dtype=np.float16)
    in_maps = []
    for c in range(C):
        m = {
            "xs_in": np.ascontiguousarray(x[c * NPC:(c + 1) * NPC]),
            "seg_in": np.ascontiguousarray(seg[c]),
            "gidx_in": gidx_w[c],
            "pool_in": np.ascontiguousarray(pool[c]),
            "id_in": ident,
        }
        for i in range(5):
            w = np.asarray(inputs[f"W{i + 1}"], dtype=np.float32)
            b = np.asarray(inputs[f"b{i + 1}"], dtype=np.float32)
            m[f"w{i}_in"] = w
            m[f"b{i}_in"] = np.ascontiguousarray(np.tile(b[None, :], (128, 1)))
        in_maps.append(m)
    return in_maps, t_pad, nchunk


def kernel(**inputs):
    in_maps, t_pad, nchunk = make_in_maps(inputs)
    nc = _get_program(t_pad, nchunk)
    res = bass_utils.run_bass_kernel_spmd(
        nc, in_maps, core_ids=list(range(C)))
    return res.results[0]["out"].astype(np.float32)



# revision 13
# speedup vs baseline: 1.7194x; 1.7194x over previous
"""GCN encoder (5-layer GCNConv + global mean pool) on 8 Trainium2 NeuronCores.

Strategy (node sharding, v2):
  - 10000 nodes split contiguously across 8 cores (1250/core, padded 1280).
  - Edges (incl. self-loops) bucketed by (dst core, dst tile of 128,
    src half); src half = first/second 640 local rows of the src's core.
    One shared Seg (GCN norm folded into one-hot values) + gather-index
    table drives the aggregation matmuls of ALL 5 layers. Buckets are
    ragged (per-bucket chunk counts); idx slots beyond the max-core valid
    count are -1 (SWDGE skips them).
  - Layer 1 aggregates x directly from replicated fp32 x tables
    (ExternalInput, half-major layout) - no collective. Messages are cast
    fp16 on the Activation engine before the one-hot matmuls.
  - Layers 2-5: per-tile GEMM (h @ W, fp16, PSUM fp32) -> fp16 cast (ACT)
    -> per-half bounce -> per-half AllGather into a 5120-row shared table
    -> SWDGE gathers (prepare_only + trigger_dma, 2 queues, pipelined) ->
    one-hot matmul aggregation. The h0 AllGather overlaps the previous
    layer's tail; the h1 AllGather overlaps the h0 aggregation pass.
    Bias folds into the h0 partial copy (DVE); relu+cast runs on ACT;
    transposes on PE.
  - Mean-pool as matmul with 1/count one-hot, AllReduce over cores.

Graph structure (edge sort, Seg with norm, gather indices, pool matrix)
is preprocessed on host; all FLOPs on x/W run on device.
"""
import sys

import numpy as np

sys.path.insert(0, "/opt/trn_rl_repo")

import concourse.bacc as bacc
import concourse.bass as bass  # noqa: F401
import concourse.mybir as mybir
import concourse.tile as tile
from concourse import bass_utils

dt = mybir.dt
AF = mybir.ActivationFunctionType

N = 10000
E = 150000
G = 64
C = 8
DIN = 128
DHID = 512
DOUT = 128
NPC = N // C          # 1250 nodes per core
NTILE = 10            # tiles of 128 dst nodes per core
HROWS = C * 640       # 5120 rows per half table
FO = [DHID, DHID, DHID, DHID, DOUT]   # per-layer GEMM output widths
FIT = [1, 4, 4, 4, 4]                 # fi tiles per GEMM


def _preprocess(edge_index, batch):
    """Bucket edges by (src half, dst tile) per dst core; build Seg, gather
    indices (half-local rows, -1 padded), and the pool matrix."""
    src = np.concatenate([edge_index[0].astype(np.int64), np.arange(N, dtype=np.int64)])
    dst = np.concatenate([edge_index[1].astype(np.int64), np.arange(N, dtype=np.int64)])
    deg = np.bincount(dst, minlength=N).astype(np.float64)
    dinv = np.where(deg > 0, 1.0 / np.sqrt(deg), 0.0)
    norm = (dinv[src] * dinv[dst]).astype(np.float32)

    core = dst // NPC
    t_of = (dst % NPC) // 128
    dloc = (dst % NPC) % 128

    s_core = src // NPC
    s_loc = src % NPC
    s_half = (s_loc >= 640).astype(np.int64)
    s_row = s_core * 640 + (s_loc - s_half * 640)   # row within half table

    # bucket key, h-major: (src half, dst tile) within each dst core
    hb = s_half * NTILE + t_of          # 0..19
    cnt = np.zeros((C, 2 * NTILE), np.int64)
    np.add.at(cnt, (core, hb), 1)
    mreg = cnt.max(axis=0)              # uniform valid count per bucket
    kp = np.maximum((mreg + 127) // 128, 1)   # chunks per bucket
    base = np.zeros(2 * NTILE, np.int64)
    base[1:] = np.cumsum(kp)[:-1]
    nchunk = int(kp.sum())

    order = np.lexsort((dst, hb, core))
    b_row, b_hb, b_core, b_dloc, b_norm = (
        s_row[order], hb[order], core[order], dloc[order], norm[order])
    gbucket = core[order] * (2 * NTILE) + b_hb
    start = np.zeros(C * 2 * NTILE, np.int64)
    start[1:] = np.cumsum(np.bincount(gbucket, minlength=C * 2 * NTILE))[:-1]
    pos = np.arange(len(order)) - start[gbucket]

    chunk = base[b_hb] + pos // 128
    erow = pos % 128

    gidx = np.full((C, nchunk * 128), -1, np.int16)
    seg = np.zeros((C, 128, nchunk, 128), np.float16)
    gidx[b_core, chunk * 128 + erow] = b_row.astype(np.int16)
    seg[b_core, erow, chunk, b_dloc] = b_norm.astype(np.float16)
    # pad every bucket to the uniform valid count with idx-0 fillers
    for c in range(C):
        for b in range(2 * NTILE):
            lo, hi = base[b] * 128 + cnt[c, b], base[b] * 128 + mreg[b]
            gidx[c, lo:hi] = 0

    # idx wrap: logical idx i -> partition i%16, column i//16; replicate x8
    gidx_w = np.ascontiguousarray(
        np.tile(gidx.reshape(C, -1, 16).transpose(0, 2, 1), (1, 8, 1)))

    # pool matrix [C, 128, NTILE, G]: 1/count at (node row, graph)
    gcnt = np.bincount(batch, minlength=G).astype(np.float64)
    inv = (1.0 / np.maximum(gcnt, 1.0))
    pool = np.zeros((C, 128, NTILE, G), np.float16)
    nodes = np.arange(N)
    pc, pr = nodes // NPC, nodes % NPC
    pool[pc, pr % 128, pr // 128, batch] = inv[batch].astype(np.float16)

    meta = (tuple(int(v) for v in kp), tuple(int(v) for v in base),
            tuple(int(v) for v in mreg), nchunk)
    return gidx_w, seg, pool, meta


def _xtables(x):
    """Half-major replicated x tables [5120, 128] fp32 each."""
    h0 = np.zeros((HROWS, DIN), np.float32)
    h1 = np.zeros((HROWS, DIN), np.float32)
    for c in range(C):
        h0[c * 640:(c + 1) * 640] = x[c * NPC: c * NPC + 640]
        h1[c * 640: c * 640 + NPC - 640] = x[c * NPC + 640:(c + 1) * NPC]
    return h0, h1


def _build(meta):
    kp, cbase, mreg, nchunk = meta
    kpmax = max(kp)
    nc = bacc.Bacc("TRN2", target_bir_lowering=False, debug=False,
                   num_devices=C, num_swdge_queues=2)

    xt_in = [nc.dram_tensor(f"xt{h}_in", [HROWS, DIN], dt.float32,
                            kind="ExternalInput") for h in range(2)]
    w_in = [nc.dram_tensor(f"w{i}_in", [DIN if i == 0 else DHID, FO[i]],
                           dt.float32, kind="ExternalInput") for i in range(5)]
    b_in = [nc.dram_tensor(f"b{i}_in", [128, FO[i]], dt.float32,
                           kind="ExternalInput") for i in range(5)]
    seg_in = nc.dram_tensor("seg_in", [128, nchunk, 128], dt.float16,
                            kind="ExternalInput")
    gidx_in = nc.dram_tensor("gidx_in", [128, nchunk * 8], dt.int16,
                             kind="ExternalInput")
    pool_in = nc.dram_tensor("pool_in", [128, NTILE, G], dt.float16,
                             kind="ExternalInput")
    id_in = nc.dram_tensor("id_in", [128, 128], dt.float16, kind="ExternalInput")
    out = nc.dram_tensor("out", [G, DOUT], dt.float32, kind="ExternalOutput")

    gshA = [nc.dram_tensor(f"gshA{h}", [HROWS, DHID], dt.float16,
                           addr_space="Shared") for h in range(2)]
    gshB = [nc.dram_tensor(f"gshB{h}", [HROWS, DOUT], dt.float16,
                           addr_space="Shared") for h in range(2)]
    bounceA = [nc.dram_tensor(f"bounceA{h}", [640, DHID], dt.float16)
               for h in range(2)]
    bounceB = [nc.dram_tensor(f"bounceB{h}", [640, DOUT], dt.float16)
               for h in range(2)]
    pool_sh = nc.dram_tensor("pool_sh", [G, DOUT], dt.float32, addr_space="Shared")
    pool_bounce = nc.dram_tensor("pool_bounce", [G, DOUT], dt.float32)

    mset_count = {}

    with tile.TileContext(nc) as tc:
        with (
            tc.tile_pool(name="const", bufs=1) as cp,
            tc.tile_pool(name="work", bufs=2) as wp,
            tc.tile_pool(name="msgp", bufs=3) as mp,
            tc.tile_pool(name="gemm_ps", bufs=2, space="PSUM") as gps,
            tc.tile_pool(name="agg_ps", bufs=2, space="PSUM") as aps,
            tc.tile_pool(name="tp_ps", bufs=2, space="PSUM") as tps,
            tc.tile_pool(name="pool_ps", bufs=1, space="PSUM") as pps,
        ):
            # ---- resident tensors ----
            seg_sb = cp.tile([128, nchunk, 128], dt.float16)
            nc.sync.dma_start(out=seg_sb[:, :, :], in_=seg_in[:, :, :])
            gidx_sb = cp.tile([128, nchunk * 8], dt.int16)
            nc.sync.dma_start(out=gidx_sb[:, :], in_=gidx_in[:, :])
            pool_sb = cp.tile([128, NTILE, G], dt.float16)
            nc.sync.dma_start(out=pool_sb[:, :, :], in_=pool_in[:, :, :])
            id16 = cp.tile([128, 128], dt.float16)
            nc.sync.dma_start(out=id16[:, :], in_=id_in[:, :])
            breps = cp.tile([128, 4, DHID], dt.float32)
            for l in range(4):
                nc.sync.dma_start(out=breps[:, l, :], in_=b_in[l][:, :])
            brep5 = cp.tile([128, DOUT], dt.float32)
            nc.sync.dma_start(out=brep5[:, :], in_=b_in[4][:, :])

            # weights -> fp16 tiles. slots: W1 -> w16[:,0]; W2..W4 -> 1+4(i-1)+j
            w16 = cp.tile([128, 13, DHID], dt.float16)
            w516 = cp.tile([128, 4, DOUT], dt.float16)
            for i in range(5):
                for j in range(FIT[i]):
                    wstage = wp.tile([128, FO[i]], dt.float32, tag="wstage")
                    nc.sync.dma_start(
                        out=wstage[:, :], in_=w_in[i][j * 128:(j + 1) * 128, :])
                    if i < 4:
                        nc.vector.tensor_copy(
                            w16[:, (0 if i == 0 else 1 + 4 * (i - 1)) + j, :],
                            wstage[:, :])
                    else:
                        nc.vector.tensor_copy(w516[:, j, :], wstage[:, :])

            hT = cp.tile([128, NTILE, 4, 128], dt.float16)
            h_out = cp.tile([128, NTILE, DOUT], dt.float16)
            partial = cp.tile([128, NTILE, DHID], dt.float32)
            pp = pps.tile([64, DOUT], dt.float32)

            def gather_bucket(l, h, t, bi):
                """Prep+trigger the SWDGE gather for bucket (h,t)."""
                q = bi % 2
                b = h * NTILE + t
                if l == 1:
                    mtile = mp.tile([128, kpmax, DIN], dt.float32, tag="m32")
                    src, esz = xt_in[h], DIN
                elif l == 5:
                    mtile = mp.tile([128, kpmax, DOUT], dt.float16, tag="mB")
                    src, esz = gshB[h], DOUT
                else:
                    mtile = mp.tile([128, kpmax, DHID], dt.float16, tag="mA")
                    src, esz = gshA[h], DHID
                # zero the skipped -1 tail slots: the gather leaves them
                # unwritten, and NaN garbage there would poison the Seg-0
                # matmul columns (0*NaN=NaN)
                kc = mreg[b] // 128
                if kc < kp[b]:
                    nc.vector.memset(
                        mtile[:, kc:kp[b], :].rearrange("p a b -> p (a b)"),
                        0.0)
                # split the bucket across both SWDGE queues so the two
                # transfers overlap (one queue's ring serializes batches)
                ka = (kp[b] + 1) // 2
                for s, (k0, k1) in enumerate(((0, ka), (ka, kp[b]))):
                    if k1 <= k0:
                        continue
                    nreg = min(mreg[b], k1 * 128) - min(mreg[b], k0 * 128)
                    if nreg <= 0:
                        continue
                    nc.gpsimd.dma_gather(
                        out_ap=mtile[:, k0:k1, :],
                        in_ap=src[:, :],
                        idxs_ap=gidx_sb[:, (cbase[b] + k0) * 8:
                                        (cbase[b] + k1) * 8],
                        num_idxs=(k1 - k0) * 128,
                        num_idxs_reg=nreg,
                        elem_size=esz,
                        single_packet=False,
                        queue_num=(q + s) % 2,
                    )
                return mtile

            def agg_bucket(l, h, t, mtile):
                """One-hot matmul accumulation of bucket (h,t) into a fresh
                PSUM aggregator; returns the aggregator tile."""
                b = h * NTILE + t
                fo = DIN if l == 1 else FO[l - 1]
                if l == 1:
                    m16 = mp.tile([128, kpmax, DIN], dt.float16, tag="m16")
                    nc.scalar.activation(
                        m16[:, :kp[b], :].rearrange("p a b -> p (a b)"),
                        mtile[:, :kp[b], :].rearrange("p a b -> p (a b)"),
                        AF.Copy)
                    mtile = m16
                pa = aps.tile([128, DHID], dt.float32, tag="pa")
                for k in range(kp[b]):
                    nc.tensor.matmul(
                        pa[:, :fo], seg_sb[:, cbase[b] + k, :], mtile[:, k, :],
                        start=(k == 0), stop=(k == kp[b] - 1))
                return pa

            def post_tile(l, t):
                """After both halves of tile t are aggregated for layer l:
                finish the tile and stage the next layer's table."""
                fo = DIN if l == 1 else FO[l - 1]
                hsum = wp.tile([128, fo], dt.float32, tag=f"hsum{fo}")
                nc.vector.tensor_tensor(
                    hsum[:, :], post_tile.pa[:, :fo], partial[:, t, :fo],
                    mybir.AluOpType.add)
                if l == 1:
                    # (Ax) -> fp16 -> transpose -> @W1 + b1 -> relu
                    st16 = wp.tile([128, DIN], dt.float16, tag="st16")
                    nc.scalar.activation(st16[:, :], hsum[:, :], AF.Copy)
                    pt1 = tps.tile([128, DHID], dt.float16, tag="pt")
                    nc.tensor.transpose(pt1[:, :128], st16[:, :], id16[:, :])
                    xT = wp.tile([128, DIN], dt.float16, tag="xT")
                    nc.vector.tensor_copy(xT[:, :], pt1[:, :128])
                    pg = gps.tile([128, DHID], dt.float32, tag="pg")
                    nc.tensor.matmul(pg[:, :], xT[:, :], w16[:, 0, :],
                                     start=True, stop=True)
                    hs2 = wp.tile([128, DHID], dt.float32, tag="hsum512")
                    nc.vector.tensor_tensor(
                        hs2[:, :], pg[:, :], breps[:, 0, :],
                        mybir.AluOpType.add)
                    hnm = wp.tile([128, DHID], dt.float16, tag="hnm")
                    nc.scalar.activation(hnm[:, :], hs2[:, :], AF.Relu)
                elif l < 5:
                    hnm = wp.tile([128, fo], dt.float16, tag="hnm")
                    nc.scalar.activation(hnm[:, :], hsum[:, :], AF.Relu)
                else:
                    nc.scalar.activation(h_out[:, t, :], hsum[:, :], AF.Relu)
                    nc.tensor.matmul(
                        pp[:, :], pool_sb[:, t, :64], h_out[:, t, :],
                        start=(t == 0), stop=(t == NTILE - 1))
                    return

                # transposes -> hT -> GEMM W_{l+1} -> cast -> bounce half
                fon = FO[l]
                bounce = bounceA if l < 4 else bounceB
                pt = tps.tile([128, DHID], dt.float16, tag="pt")
                for j in range(4):
                    nc.tensor.transpose(
                        pt[:, j * 128:(j + 1) * 128],
                        hnm[:, j * 128:(j + 1) * 128], id16[:, :])
                nc.vector.tensor_copy(
                    hT[:, t, :, :].rearrange("p a b -> p (a b)"), pt[:, :512])
                pg2 = gps.tile([128, fon], dt.float32, tag="pg")
                for j in range(4):
                    wslot = (w16[:, 1 + 4 * (l - 1) + j, :] if l < 4
                             else w516[:, j, :])
                    nc.tensor.matmul(pg2[:, :], hT[:, t, j, :], wslot,
                                     start=(j == 0), stop=(j == 3))
                hw16 = wp.tile([128, fon], dt.float16, tag="hw16")
                nc.scalar.activation(hw16[:, :], pg2[:, :], AF.Copy)
                hh, r = t // 5, (t % 5) * 128
                nc.sync.dma_start(out=bounce[hh].ap()[r:r + 128, :],
                                  in_=hw16[:, :])

            def ag_half(bounce, gsh):
                nc.gpsimd.collective_compute(
                    "AllGather", mybir.AluOpType.bypass,
                    replica_groups=[list(range(C))],
                    ins=[bounce.ap().opt()],
                    outs=[gsh.ap().opt()])

            # ========================= LAYERS 1..5 =========================
            bi = 0
            for l in range(1, 6):
                fo = DIN if l == 1 else FO[l - 1]
                for h in range(2):
                    for t in range(NTILE):
                        mt = gather_bucket(l, h, t, bi); bi += 1
                        pa = agg_bucket(l, h, t, mt)
                        if h == 0:
                            # fold the post-agg bias into the partial copy
                            if l == 1:
                                nc.vector.tensor_copy(
                                    partial[:, t, :fo], pa[:, :fo])
                            else:
                                nc.vector.tensor_tensor(
                                    partial[:, t, :fo], pa[:, :fo],
                                    breps[:, l - 1, :fo] if l < 5
                                    else brep5[:, :],
                                    mybir.AluOpType.add)
                        else:
                            post_tile.pa = pa
                            post_tile(l, t)
                            if l < 5:
                                bounce = bounceA if l < 4 else bounceB
                                gsh = gshA if l < 4 else gshB
                                if t == 4:
                                    ag_half(bounce[0], gsh[0])
                                if t == 9:
                                    ag_half(bounce[1], gsh[1])

            # ---- mean pool: AllReduce over cores ----
            pres = wp.tile([64, DOUT], dt.float32, tag="pres")
            nc.vector.tensor_copy(pres[:, :], pp[:, :])
            nc.sync.dma_start(out=pool_bounce[:, :], in_=pres[:, :])
            nc.gpsimd.collective_compute(
                "AllReduce", mybir.AluOpType.add,
                replica_groups=[list(range(C))],
                ins=[pool_bounce.ap().opt()],
                outs=[pool_sh.ap().opt()])
            ores = wp.tile([64, DOUT], dt.float32, tag="ores")
            nc.sync.dma_start(out=ores[:, :], in_=pool_sh[:, :])
            nc.sync.dma_start(out=out[:, :], in_=ores[:, :])

    nc.compile()
    return nc


_CACHE = {}


def _get_program(meta):
    if meta not in _CACHE:
        _CACHE[meta] = _build(meta)
    return _CACHE[meta]


def make_in_maps(inputs):
    edge_index = np.asarray(inputs["edge_index"])
    batch = np.asarray(inputs["batch"])
    x = np.asarray(inputs["x"], dtype=np.float32)
    gidx_w, seg, pool, meta = _preprocess(edge_index, batch)
    xt0, xt1 = _xtables(x)
    ident = np.eye(128, dtype=np.float16)
    in_maps = []
    for c in range(C):
        m = {
            "xt0_in": xt0,
            "xt1_in": xt1,
            "seg_in": np.ascontiguousarray(seg[c]),
            "gidx_in": gidx_w[c],
            "pool_in": np.ascontiguousarray(pool[c]),
            "id_in": ident,
        }
        for i in range(5):
            w = np.asarray(inputs[f"W{i + 1}"], dtype=np.float32)
            b = np.asarray(inputs[f"b{i + 1}"], dtype=np.float32)
            m[f"w{i}_in"] = w
            m[f"b{i}_in"] = np.ascontiguousarray(np.tile(b[None, :], (128, 1)))
        in_maps.append(m)
    return in_maps, meta


def kernel(**inputs):
    in_maps, meta = make_in_maps(inputs)
    nc = _get_program(meta)
    res = bass_utils.run_bass_kernel_spmd(
        nc, in_maps, core_ids=list(range(C)))
    return res.results[0]["out"].astype(np.float32)


# revision 20
# speedup vs baseline: 1.8673x; 1.0860x over previous
"""GCN encoder (5-layer GCNConv + global mean pool) on 8 Trainium2 NeuronCores.

Strategy (node sharding, v2):
  - 10000 nodes split contiguously across 8 cores (1250/core, padded 1280).
  - Edges (incl. self-loops) bucketed by (dst core, dst tile of 128,
    src half); src half = first/second 640 local rows of the src's core.
    One shared Seg (GCN norm folded into one-hot values) + gather-index
    table drives the aggregation matmuls of ALL 5 layers. Buckets are
    ragged (per-bucket chunk counts); idx slots beyond the max-core valid
    count are -1 (SWDGE skips them).
  - Layer 1 aggregates x directly from replicated fp32 x tables
    (ExternalInput, half-major layout) - no collective. Messages are cast
    fp16 on the Activation engine before the one-hot matmuls.
  - Layers 2-5: per-tile GEMM (h @ W, fp16, PSUM fp32) -> fp16 cast (ACT)
    -> per-half bounce -> per-half AllGather into a 5120-row shared table
    -> SWDGE gathers (prepare_only + trigger_dma, 2 queues, pipelined) ->
    one-hot matmul aggregation. The h0 AllGather overlaps the previous
    layer's tail; the h1 AllGather overlaps the h0 aggregation pass.
    Bias folds into the h0 partial copy (DVE); relu+cast runs on ACT;
    transposes on PE.
  - Mean-pool as matmul with 1/count one-hot, AllReduce over cores.

Graph structure (edge sort, Seg with norm, gather indices, pool matrix)
is preprocessed on host; all FLOPs on x/W run on device.
"""
import sys

import numpy as np

sys.path.insert(0, "/opt/trn_rl_repo")

import concourse.bacc as bacc
import concourse.bass as bass  # noqa: F401
import concourse.mybir as mybir
import concourse.tile as tile
from concourse import bass_utils

dt = mybir.dt
AF = mybir.ActivationFunctionType

N = 10000
E = 150000
G = 64
C = 8
DIN = 128
DHID = 512
DOUT = 128
NPC = N // C          # 1250 nodes per core
NTILE = 10            # tiles of 128 dst nodes per core
HROWS = C * 640       # 5120 rows per half table
FO = [DHID, DHID, DHID, DHID, DOUT]   # per-layer GEMM output widths
FIT = [1, 4, 4, 4, 4]                 # fi tiles per GEMM


def _preprocess(edge_index, batch):
    """Bucket edges by (src half, dst tile) per dst core; build Seg, gather
    indices (half-local rows, -1 padded), and the pool matrix."""
    src = np.concatenate([edge_index[0].astype(np.int64), np.arange(N, dtype=np.int64)])
    dst = np.concatenate([edge_index[1].astype(np.int64), np.arange(N, dtype=np.int64)])
    deg = np.bincount(dst, minlength=N).astype(np.float64)
    dinv = np.where(deg > 0, 1.0 / np.sqrt(deg), 0.0)
    norm = (dinv[src] * dinv[dst]).astype(np.float32)

    core = dst // NPC
    t_of = (dst % NPC) // 128
    dloc = (dst % NPC) % 128

    s_core = src // NPC
    s_loc = src % NPC
    s_half = (s_loc >= 640).astype(np.int64)
    s_row = s_core * 640 + (s_loc - s_half * 640)   # row within half table

    # bucket key, h-major: (src half, dst tile) within each dst core.
    # Duplicate src rows within a bucket share one gather slot (the Seg
    # column for that slot is multi-hot with summed norms).
    hb = s_half * NTILE + t_of          # 0..19
    order = np.lexsort((s_row, hb, core))
    b_row, b_hb, b_core, b_dloc, b_norm = (
        s_row[order], hb[order], core[order], dloc[order], norm[order])
    gbucket = b_core * (2 * NTILE) + b_hb
    new_grp = np.ones(len(order), bool)
    new_grp[1:] = (gbucket[1:] != gbucket[:-1]) | (b_row[1:] != b_row[:-1])
    uid = np.cumsum(new_grp) - 1
    bnd = np.ones(len(order), bool)
    bnd[1:] = gbucket[1:] != gbucket[:-1]
    start_uid = np.maximum.accumulate(np.where(bnd, uid, -1))
    slot = uid - start_uid

    ucnt = np.zeros((C, 2 * NTILE), np.int64)
    np.add.at(ucnt, (b_core, b_hb), new_grp)
    mreg = ucnt.max(axis=0)             # uniform valid slot count per bucket
    kp = np.maximum((mreg + 127) // 128, 1)   # chunks per bucket
    base = np.zeros(2 * NTILE, np.int64)
    base[1:] = np.cumsum(kp)[:-1]
    nchunk = int(kp.sum())

    chunk = base[b_hb] + slot // 128
    erow = slot % 128

    gidx = np.full((C, nchunk * 128), -1, np.int16)
    seg32 = np.zeros((C, 128, nchunk, 128), np.float32)
    gidx[b_core, chunk * 128 + erow] = b_row.astype(np.int16)
    np.add.at(seg32, (b_core, erow, chunk, b_dloc), b_norm)
    seg = seg32.astype(np.float16)
    # pad every bucket to the uniform valid count with idx-0 fillers
    for c in range(C):
        for b in range(2 * NTILE):
            lo, hi = base[b] * 128 + ucnt[c, b], base[b] * 128 + mreg[b]
            gidx[c, lo:hi] = 0

    # idx wrap: logical idx i -> partition i%16, column i//16; replicate x8
    gidx_w = np.ascontiguousarray(
        np.tile(gidx.reshape(C, -1, 16).transpose(0, 2, 1), (1, 8, 1)))

    # pool matrix [C, 128, NTILE, G]: 1/count at (node row, graph)
    gcnt = np.bincount(batch, minlength=G).astype(np.float64)
    inv = (1.0 / np.maximum(gcnt, 1.0))
    pool = np.zeros((C, 128, NTILE, G), np.float16)
    nodes = np.arange(N)
    pc, pr = nodes // NPC, nodes % NPC
    pool[pc, pr % 128, pr // 128, batch] = inv[batch].astype(np.float16)

    meta = (tuple(int(v) for v in kp), tuple(int(v) for v in base),
            tuple(int(v) for v in mreg), nchunk)
    return gidx_w, seg, pool, meta


def _xtables(x):
    """Half-major replicated x tables [5120, 128] fp32 each."""
    h0 = np.zeros((HROWS, DIN), np.float32)
    h1 = np.zeros((HROWS, DIN), np.float32)
    for c in range(C):
        h0[c * 640:(c + 1) * 640] = x[c * NPC: c * NPC + 640]
        h1[c * 640: c * 640 + NPC - 640] = x[c * NPC + 640:(c + 1) * NPC]
    return h0, h1


def _build(meta):
    kp, cbase, mreg, nchunk = meta
    kpmax = max(kp)
    nc = bacc.Bacc("TRN2", target_bir_lowering=False, debug=False,
                   num_devices=C, num_swdge_queues=4)

    xt_in = [nc.dram_tensor(f"xt{h}_in", [HROWS, DIN], dt.float32,
                            kind="ExternalInput") for h in range(2)]
    w_in = [nc.dram_tensor(f"w{i}_in", [DIN if i == 0 else DHID, FO[i]],
                           dt.float32, kind="ExternalInput") for i in range(5)]
    b_in = [nc.dram_tensor(f"b{i}_in", [128, FO[i]], dt.float32,
                           kind="ExternalInput") for i in range(5)]
    seg_in = nc.dram_tensor("seg_in", [128, nchunk, 128], dt.float16,
                            kind="ExternalInput")
    gidx_in = nc.dram_tensor("gidx_in", [128, nchunk * 8], dt.int16,
                             kind="ExternalInput")
    pool_in = nc.dram_tensor("pool_in", [128, NTILE, G], dt.float16,
                             kind="ExternalInput")
    id_in = nc.dram_tensor("id_in", [128, 128], dt.float16, kind="ExternalInput")
    out = nc.dram_tensor("out", [G, DOUT], dt.float32, kind="ExternalOutput")

    gshA = [nc.dram_tensor(f"gshA{h}", [HROWS, DHID], dt.float16,
                           addr_space="Shared") for h in range(2)]
    gshB = [nc.dram_tensor(f"gshB{h}", [HROWS, DOUT], dt.float16,
                           addr_space="Shared") for h in range(2)]
    bounceA = [nc.dram_tensor(f"bounceA{h}", [640, DHID], dt.float16)
               for h in range(2)]
    bounceB = [nc.dram_tensor(f"bounceB{h}", [640, DOUT], dt.float16)
               for h in range(2)]
    pool_sh = nc.dram_tensor("pool_sh", [G, DOUT], dt.float32, addr_space="Shared")
    pool_bounce = nc.dram_tensor("pool_bounce", [G, DOUT], dt.float32)

    mset_count = {}

    with tile.TileContext(nc) as tc:
        with (
            tc.tile_pool(name="const", bufs=1) as cp,
            tc.tile_pool(name="work", bufs=2) as wp,
            tc.tile_pool(name="msgp", bufs=4) as mp,
            tc.tile_pool(name="gemm_ps", bufs=2, space="PSUM") as gps,
            tc.tile_pool(name="agg_ps", bufs=2, space="PSUM") as aps,
            tc.tile_pool(name="tp_ps", bufs=2, space="PSUM") as tps,
            tc.tile_pool(name="pool_ps", bufs=1, space="PSUM") as pps,
        ):
            # ---- resident tensors (gidx first: it gates the first gather) ----
            gidx_sb = cp.tile([128, nchunk * 8], dt.int16)
            nc.sync.dma_start(out=gidx_sb[:, :], in_=gidx_in[:, :])
            seg_sb = cp.tile([128, nchunk, 128], dt.float16)
            c10 = cbase[NTILE]       # first h1 chunk: split the load so the
            nc.sync.dma_start(       # h0 matmuls aren't gated on the full 11MB
                out=seg_sb[:, :c10, :], in_=seg_in[:, :c10, :])
            nc.sync.dma_start(
                out=seg_sb[:, c10:, :], in_=seg_in[:, c10:, :])
            pool_sb = cp.tile([128, NTILE, G], dt.float16)
            nc.sync.dma_start(out=pool_sb[:, :, :], in_=pool_in[:, :, :])
            id16 = cp.tile([128, 128], dt.float16)
            nc.sync.dma_start(out=id16[:, :], in_=id_in[:, :])
            breps = cp.tile([128, 4, DHID], dt.float32)
            for l in range(4):
                nc.sync.dma_start(out=breps[:, l, :], in_=b_in[l][:, :])
            brep5 = cp.tile([128, DOUT], dt.float32)
            nc.sync.dma_start(out=brep5[:, :], in_=b_in[4][:, :])

            # weights -> fp16 tiles. slots: W1 -> w16[:,0]; W2..W4 -> 1+4(i-1)+j
            w16 = cp.tile([128, 13, DHID], dt.float16)
            w516 = cp.tile([128, 4, DOUT], dt.float16)
            for i in range(5):
                for j in range(FIT[i]):
                    wstage = wp.tile([128, FO[i]], dt.float32, tag="wstage")
                    nc.sync.dma_start(
                        out=wstage[:, :], in_=w_in[i][j * 128:(j + 1) * 128, :])
                    if i < 4:
                        nc.vector.tensor_copy(
                            w16[:, (0 if i == 0 else 1 + 4 * (i - 1)) + j, :],
                            wstage[:, :])
                    else:
                        nc.vector.tensor_copy(w516[:, j, :], wstage[:, :])

            hT = cp.tile([128, NTILE, 4, 128], dt.float16)
            h_out = cp.tile([128, NTILE, DOUT], dt.float16)
            partial = cp.tile([128, NTILE, DHID], dt.float16)
            pp = pps.tile([64, DOUT], dt.float32)

            def gather_bucket(l, h, t, bi):
                """Prep+trigger the SWDGE gather for bucket (h,t)."""
                q = (2 * bi) % 4
                b = h * NTILE + t
                if l == 1:
                    mtile = mp.tile([128, kpmax, DIN], dt.float32, tag="m32")
                    src, esz = xt_in[h], DIN
                elif l == 5:
                    mtile = mp.tile([128, kpmax, DOUT], dt.float16, tag="mB")
                    src, esz = gshB[h], DOUT
                else:
                    mtile = mp.tile([128, kpmax, DHID], dt.float16, tag="mA")
                    src, esz = gshA[h], DHID
                # zero the skipped -1 tail slots: the gather leaves them
                # unwritten, and NaN garbage there would poison the Seg-0
                # matmul columns (0*NaN=NaN)
                kc = mreg[b] // 128
                if kc < kp[b]:
                    nc.vector.memset(
                        mtile[:, kc:kp[b], :].rearrange("p a b -> p (a b)"),
                        0.0)
                # split the bucket across both SWDGE queues so the two
                # transfers overlap (one queue's ring serializes batches)
                ka = (kp[b] + 1) // 2
                for s, (k0, k1) in enumerate(((0, ka), (ka, kp[b]))):
                    if k1 <= k0:
                        continue
                    nreg = min(mreg[b], k1 * 128) - min(mreg[b], k0 * 128)
                    if nreg <= 0:
                        continue
                    nc.gpsimd.dma_gather(
                        out_ap=mtile[:, k0:k1, :],
                        in_ap=src[:, :],
                        idxs_ap=gidx_sb[:, (cbase[b] + k0) * 8:
                                        (cbase[b] + k1) * 8],
                        num_idxs=(k1 - k0) * 128,
                        num_idxs_reg=nreg,
                        elem_size=esz,
                        single_packet=False,
                        queue_num=(q + s) % 4,
                    )
                return mtile

            def agg_bucket(l, h, t, mtile):
                """One-hot matmul accumulation of bucket (h,t) into a fresh
                PSUM aggregator; returns the aggregator tile."""
                b = h * NTILE + t
                fo = DIN if l == 1 else FO[l - 1]
                if l == 1:
                    m16 = mp.tile([128, kpmax, DIN], dt.float16, tag="m16")
                    nc.scalar.activation(
                        m16[:, :kp[b], :].rearrange("p a b -> p (a b)"),
                        mtile[:, :kp[b], :].rearrange("p a b -> p (a b)"),
                        AF.Copy)
                    mtile = m16
                pa = aps.tile([128, DHID], dt.float32, tag="pa")
                for k in range(kp[b]):
                    nc.tensor.matmul(
                        pa[:, :fo], seg_sb[:, cbase[b] + k, :], mtile[:, k, :],
                        start=(k == 0), stop=(k == kp[b] - 1))
                return pa

            def post_tile(l, t):
                """After both halves of tile t are aggregated for layer l:
                finish the tile and stage the next layer's table."""
                fo = DIN if l == 1 else FO[l - 1]
                hsum = wp.tile([128, fo], dt.float32, tag=f"hsum{fo}")
                nc.vector.tensor_tensor(
                    hsum[:, :], post_tile.pa[:, :fo], partial[:, t, :fo],
                    mybir.AluOpType.add)
                if l == 1:
                    # (Ax) -> fp16 -> transpose -> @W1 + b1 -> relu
                    st16 = wp.tile([128, DIN], dt.float16, tag="st16")
                    nc.scalar.activation(st16[:, :], hsum[:, :], AF.Copy)
                    pt1 = tps.tile([128, DHID], dt.float16, tag="pt")
                    nc.tensor.transpose(pt1[:, :128], st16[:, :], id16[:, :])
                    xT = wp.tile([128, DIN], dt.float16, tag="xT")
                    nc.vector.tensor_copy(xT[:, :], pt1[:, :128])
                    pg = gps.tile([128, DHID], dt.float32, tag="pg")
                    nc.tensor.matmul(pg[:, :], xT[:, :], w16[:, 0, :],
                                     start=True, stop=True)
                    hs2 = wp.tile([128, DHID], dt.float32, tag="hsum512")
                    nc.vector.tensor_tensor(
                        hs2[:, :], pg[:, :], breps[:, 0, :],
                        mybir.AluOpType.add)
                    hnm = wp.tile([128, DHID], dt.float16, tag="hnm")
                    nc.scalar.activation(hnm[:, :], hs2[:, :], AF.Relu)
                elif l < 5:
                    hnm = wp.tile([128, fo], dt.float16, tag="hnm")
                    nc.scalar.activation(hnm[:, :], hsum[:, :], AF.Relu)
                else:
                    nc.scalar.activation(h_out[:, t, :], hsum[:, :], AF.Relu)
                    nc.tensor.matmul(
                        pp[:, :], pool_sb[:, t, :64], h_out[:, t, :],
                        start=(t == 0), stop=(t == NTILE - 1))
                    return

                # transposes -> hT -> GEMM W_{l+1} -> cast -> bounce half
                fon = FO[l]
                bounce = bounceA if l < 4 else bounceB
                pt = tps.tile([128, DHID], dt.float16, tag="pt")
                for j in range(4):
                    nc.tensor.transpose(
                        pt[:, j * 128:(j + 1) * 128],
                        hnm[:, j * 128:(j + 1) * 128], id16[:, :])
                nc.vector.tensor_copy(
                    hT[:, t, :, :].rearrange("p a b -> p (a b)"), pt[:, :512])
                pg2 = gps.tile([128, fon], dt.float32, tag="pg")
                for j in range(4):
                    wslot = (w16[:, 1 + 4 * (l - 1) + j, :] if l < 4
                             else w516[:, j, :])
                    nc.tensor.matmul(pg2[:, :], hT[:, t, j, :], wslot,
                                     start=(j == 0), stop=(j == 3))
                hw16 = wp.tile([128, fon], dt.float16, tag="hw16")
                nc.scalar.activation(hw16[:, :], pg2[:, :], AF.Copy)
                hh, r = t // 5, (t % 5) * 128
                nc.sync.dma_start(out=bounce[hh].ap()[r:r + 128, :],
                                  in_=hw16[:, :])

            def ag_half(bounce, gsh):
                nc.gpsimd.collective_compute(
                    "AllGather", mybir.AluOpType.bypass,
                    replica_groups=[list(range(C))],
                    ins=[bounce.ap().opt()],
                    outs=[gsh.ap().opt()])

            # ========================= LAYERS 1..5 =========================
            # Pool-stream order per layer: [AG(l,h0) trigger] h0 gathers,
            # [AG(l,h1) trigger] h1 gathers. The AG triggers wait on the
            # previous layer's bounce writes; placing them at half-pass
            # heads keeps them from head-of-line-blocking gather issue.
            # post_tile lags the aggregation by one bucket so its
            # DVE->ACT->PE chain hides under the next bucket's matmuls.
            bi = 0
            for l in range(1, 6):
                fo = DIN if l == 1 else FO[l - 1]
                for h in range(2):
                    if l >= 2:
                        ag_half((bounceA if l < 5 else bounceB)[h],
                                (gshA if l < 5 else gshB)[h])
                    pend = None
                    for t in range(NTILE):
                        mt = gather_bucket(l, h, t, bi); bi += 1
                        pa = agg_bucket(l, h, t, mt)
                        if h == 0:
                            # fold the post-agg bias into the partial copy
                            if l == 1:
                                nc.vector.tensor_copy(
                                    partial[:, t, :fo], pa[:, :fo])
                            else:
                                nc.vector.tensor_tensor(
                                    partial[:, t, :fo], pa[:, :fo],
                                    breps[:, l - 1, :fo] if l < 5
                                    else brep5[:, :],
                                    mybir.AluOpType.add)
                        else:
                            if pend is not None:
                                post_tile.pa = pend[1]
                                post_tile(l, pend[0])
                            pend = (t, pa)
                    if h == 1:
                        post_tile.pa = pend[1]
                        post_tile(l, pend[0])

            # ---- mean pool: AllReduce over cores ----
            pres = wp.tile([64, DOUT], dt.float32, tag="pres")
            nc.vector.tensor_copy(pres[:, :], pp[:, :])
            nc.sync.dma_start(out=pool_bounce[:, :], in_=pres[:, :])
            nc.gpsimd.collective_compute(
                "AllReduce", mybir.AluOpType.add,
                replica_groups=[list(range(C))],
                ins=[pool_bounce.ap().opt()],
                outs=[pool_sh.ap().opt()])
            ores = wp.tile([64, DOUT], dt.float32, tag="ores")
            nc.sync.dma_start(out=ores[:, :], in_=pool_sh[:, :])
            nc.sync.dma_start(out=out[:, :], in_=ores[:, :])

    nc.compile()
    return nc


_CACHE = {}


def _get_program(meta):
    if meta not in _CACHE:
        _CACHE[meta] = _build(meta)
    return _CACHE[meta]


def make_in_maps(inputs):
    edge_index = np.asarray(inputs["edge_index"])
    batch = np.asarray(inputs["batch"])
    x = np.asarray(inputs["x"], dtype=np.float32)
    gidx_w, seg, pool, meta = _preprocess(edge_index, batch)
    xt0, xt1 = _xtables(x)
    ident = np.eye(128, dtype=np.float16)
    in_maps = []
    for c in range(C):
        m = {
            "xt0_in": xt0,
            "xt1_in": xt1,
            "seg_in": np.ascontiguousarray(seg[c]),
            "gidx_in": gidx_w[c],
            "pool_in": np.ascontiguousarray(pool[c]),
            "id_in": ident,
        }
        for i in range(5):
            w = np.asarray(inputs[f"W{i + 1}"], dtype=np.float32)
            b = np.asarray(inputs[f"b{i + 1}"], dtype=np.float32)
            m[f"w{i}_in"] = w
            m[f"b{i}_in"] = np.ascontiguousarray(np.tile(b[None, :], (128, 1)))
        in_maps.append(m)
    return in_maps, meta


def kernel(**inputs):
    in_maps, meta = make_in_maps(inputs)
    nc = _get_program(meta)
    res = bass_utils.run_bass_kernel_spmd(
        nc, in_maps, core_ids=list(range(C)))
    return res.results[0]["out"].astype(np.float32)


# revision 26
# speedup vs baseline: 2.0027x; 1.0725x over previous
"""GCN encoder (5-layer GCNConv + global mean pool) on 8 Trainium2 NeuronCores.

Strategy (node sharding, v2):
  - 10000 nodes split contiguously across 8 cores (1250/core, padded 1280).
  - Edges (incl. self-loops) bucketed by (dst core, dst tile of 128,
    src half); src half = first/second 640 local rows of the src's core.
    One shared Seg (GCN norm folded into one-hot values) + gather-index
    table drives the aggregation matmuls of ALL 5 layers. Buckets are
    ragged (per-bucket chunk counts); idx slots beyond the max-core valid
    count are -1 (SWDGE skips them).
  - Layer 1 aggregates x directly from replicated fp32 x tables
    (ExternalInput, half-major layout) - no collective. Messages are cast
    fp16 on the Activation engine before the one-hot matmuls.
  - Layers 2-5: per-tile GEMM (h @ W, fp16, PSUM fp32) -> fp16 cast (ACT)
    -> per-half bounce -> per-half AllGather into a 5120-row shared table
    -> SWDGE gathers (prepare_only + trigger_dma, 2 queues, pipelined) ->
    one-hot matmul aggregation. The h0 AllGather overlaps the previous
    layer's tail; the h1 AllGather overlaps the h0 aggregation pass.
    Bias folds into the h0 partial copy (DVE); relu+cast runs on ACT;
    transposes on PE.
  - Mean-pool as matmul with 1/count one-hot, AllReduce over cores.

Graph structure (edge sort, Seg with norm, gather indices, pool matrix)
is preprocessed on host; all FLOPs on x/W run on device.
"""
import sys

import numpy as np

sys.path.insert(0, "/opt/trn_rl_repo")

import concourse.bacc as bacc
import concourse.bass as bass  # noqa: F401
import concourse.mybir as mybir
import concourse.tile as tile
from concourse import bass_utils

dt = mybir.dt
AF = mybir.ActivationFunctionType

N = 10000
E = 150000
G = 64
C = 8
DIN = 128
DHID = 512
DOUT = 128
NPC = N // C          # 1250 nodes per core
NTILE = 10            # tiles of 128 dst nodes per core
HROWS = C * 640       # 5120 rows per half table
FO = [DHID, DHID, DHID, DHID, DOUT]   # per-layer GEMM output widths
FIT = [1, 4, 4, 4, 4]                 # fi tiles per GEMM


def _preprocess(edge_index, batch):
    """Bucket edges by (src half, dst tile) per dst core; build Seg, gather
    indices (half-local rows, -1 padded), and the pool matrix."""
    src = np.concatenate([edge_index[0].astype(np.int64), np.arange(N, dtype=np.int64)])
    dst = np.concatenate([edge_index[1].astype(np.int64), np.arange(N, dtype=np.int64)])
    deg = np.bincount(dst, minlength=N).astype(np.float64)
    dinv = np.where(deg > 0, 1.0 / np.sqrt(deg), 0.0)
    norm = (dinv[src] * dinv[dst]).astype(np.float32)

    core = dst // NPC
    t_of = (dst % NPC) // 128
    dloc = (dst % NPC) % 128

    s_core = src // NPC
    s_loc = src % NPC
    s_half = (s_loc >= 640).astype(np.int64)
    s_row = s_core * 640 + (s_loc - s_half * 640)   # row within half table

    # bucket key, h-major: (src half, dst tile) within each dst core.
    # Duplicate src rows within a bucket share one gather slot (the Seg
    # column for that slot is multi-hot with summed norms).
    hb = s_half * NTILE + t_of          # 0..19
    order = np.lexsort((s_row, hb, core))
    b_row, b_hb, b_core, b_dloc, b_norm = (
        s_row[order], hb[order], core[order], dloc[order], norm[order])
    gbucket = b_core * (2 * NTILE) + b_hb
    new_grp = np.ones(len(order), bool)
    new_grp[1:] = (gbucket[1:] != gbucket[:-1]) | (b_row[1:] != b_row[:-1])
    uid = np.cumsum(new_grp) - 1
    bnd = np.ones(len(order), bool)
    bnd[1:] = gbucket[1:] != gbucket[:-1]
    start_uid = np.maximum.accumulate(np.where(bnd, uid, -1))
    slot = uid - start_uid

    ucnt = np.zeros((C, 2 * NTILE), np.int64)
    np.add.at(ucnt, (b_core, b_hb), new_grp)
    mreg = ucnt.max(axis=0)             # uniform valid slot count per bucket
    kp = np.maximum((mreg + 127) // 128, 1)   # chunks per bucket
    base = np.zeros(2 * NTILE, np.int64)
    base[1:] = np.cumsum(kp)[:-1]
    nchunk = int(kp.sum())

    chunk = base[b_hb] + slot // 128
    erow = slot % 128

    gidx = np.full((C, nchunk * 128), -1, np.int16)
    seg32 = np.zeros((C, 128, nchunk, 128), np.float32)
    gidx[b_core, chunk * 128 + erow] = b_row.astype(np.int16)
    np.add.at(seg32, (b_core, erow, chunk, b_dloc), b_norm)
    seg = seg32.astype(np.float16)
    # pad every bucket to the uniform valid count with idx-0 fillers
    for c in range(C):
        for b in range(2 * NTILE):
            lo, hi = base[b] * 128 + ucnt[c, b], base[b] * 128 + mreg[b]
            gidx[c, lo:hi] = 0

    # idx wrap: logical idx i -> partition i%16, column i//16; replicate x8
    gidx_w = np.ascontiguousarray(
        np.tile(gidx.reshape(C, -1, 16).transpose(0, 2, 1), (1, 8, 1)))

    # pool matrix [C, 128, NTILE, G]: 1/count at (node row, graph)
    gcnt = np.bincount(batch, minlength=G).astype(np.float64)
    inv = (1.0 / np.maximum(gcnt, 1.0))
    pool = np.zeros((C, 128, NTILE, G), np.float16)
    nodes = np.arange(N)
    pc, pr = nodes // NPC, nodes % NPC
    pool[pc, pr % 128, pr // 128, batch] = inv[batch].astype(np.float16)

    meta = (tuple(int(v) for v in kp), tuple(int(v) for v in base),
            tuple(int(v) for v in mreg), nchunk)
    return gidx_w, seg, pool, meta


def _xtables(x):
    """Half-major replicated x tables [5120, 128] fp32 each."""
    h0 = np.zeros((HROWS, DIN), np.float32)
    h1 = np.zeros((HROWS, DIN), np.float32)
    for c in range(C):
        h0[c * 640:(c + 1) * 640] = x[c * NPC: c * NPC + 640]
        h1[c * 640: c * 640 + NPC - 640] = x[c * NPC + 640:(c + 1) * NPC]
    return h0, h1


def _build(meta):
    kp, cbase, mreg, nchunk = meta
    kpmax = max(kp)
    nc = bacc.Bacc("TRN2", target_bir_lowering=False, debug=False,
                   num_devices=C, num_swdge_queues=4)

    xt_in = [nc.dram_tensor(f"xt{h}_in", [HROWS, DIN], dt.float32,
                            kind="ExternalInput") for h in range(2)]
    w_in = [nc.dram_tensor(f"w{i}_in", [DIN if i == 0 else DHID, FO[i]],
                           dt.float32, kind="ExternalInput") for i in range(5)]
    b_in = [nc.dram_tensor(f"b{i}_in", [128, FO[i]], dt.float32,
                           kind="ExternalInput") for i in range(5)]
    seg_in = nc.dram_tensor("seg_in", [128, nchunk, 128], dt.float16,
                            kind="ExternalInput")
    gidx_in = nc.dram_tensor("gidx_in", [128, nchunk * 8], dt.int16,
                             kind="ExternalInput")
    pool_in = nc.dram_tensor("pool_in", [128, NTILE, G], dt.float16,
                             kind="ExternalInput")
    id_in = nc.dram_tensor("id_in", [128, 128], dt.float16, kind="ExternalInput")
    out = nc.dram_tensor("out", [G, DOUT], dt.float32, kind="ExternalOutput")

    gshA = [nc.dram_tensor(f"gshA{h}", [HROWS, DHID], dt.float16,
                           addr_space="Shared") for h in range(2)]
    gshB = [nc.dram_tensor(f"gshB{h}", [HROWS, DOUT], dt.float16,
                           addr_space="Shared") for h in range(2)]
    bounceA = [nc.dram_tensor(f"bounceA{h}", [640, DHID], dt.float16)
               for h in range(2)]
    bounceB = [nc.dram_tensor(f"bounceB{h}", [640, DOUT], dt.float16)
               for h in range(2)]
    pool_sh = nc.dram_tensor("pool_sh", [G, DOUT], dt.float32, addr_space="Shared")
    pool_bounce = nc.dram_tensor("pool_bounce", [G, DOUT], dt.float32)

    mset_count = {}

    with tile.TileContext(nc) as tc:
        with (
            tc.tile_pool(name="const", bufs=1) as cp,
            tc.tile_pool(name="work", bufs=2) as wp,
            tc.tile_pool(name="msgp", bufs=4) as mp,
            tc.tile_pool(name="gemm_ps", bufs=2, space="PSUM") as gps,
            tc.tile_pool(name="agg_ps", bufs=2, space="PSUM") as aps,
            tc.tile_pool(name="tp_ps", bufs=2, space="PSUM") as tps,
            tc.tile_pool(name="pool_ps", bufs=1, space="PSUM") as pps,
        ):
            # ---- resident tensors (gidx first: it gates the first gather) ----
            gidx_sb = cp.tile([128, nchunk * 8], dt.int16)
            nc.sync.dma_start(out=gidx_sb[:, :], in_=gidx_in[:, :])
            seg_sb = cp.tile([128, nchunk, 128], dt.float16)
            c10 = cbase[NTILE]       # first h1 chunk: split the load so the
            nc.sync.dma_start(       # h0 matmuls aren't gated on the full 11MB
                out=seg_sb[:, :c10, :], in_=seg_in[:, :c10, :])
            nc.sync.dma_start(
                out=seg_sb[:, c10:, :], in_=seg_in[:, c10:, :])
            pool_sb = cp.tile([128, NTILE, G], dt.float16)
            id16 = cp.tile([128, 128], dt.float16)
            nc.sync.dma_start(out=id16[:, :], in_=id_in[:, :])
            breps = cp.tile([128, 4, DHID], dt.float32)
            for l in range(2):
                nc.sync.dma_start(out=breps[:, l, :], in_=b_in[l][:, :])
            brep5 = cp.tile([128, DOUT], dt.float32)

            # weights -> fp16 tiles. slots: W1 -> w16[:,0]; W2..W4 -> 1+4(i-1)+j
            w16 = cp.tile([128, 13, DHID], dt.float16)
            w516 = cp.tile([128, 4, DOUT], dt.float16)

            def load_weight(i):
                for j in range(FIT[i]):
                    wstage = wp.tile([128, FO[i]], dt.float32, tag="wstage")
                    nc.sync.dma_start(
                        out=wstage[:, :], in_=w_in[i][j * 128:(j + 1) * 128, :])
                    if i < 4:
                        nc.vector.tensor_copy(
                            w16[:, (0 if i == 0 else 1 + 4 * (i - 1)) + j, :],
                            wstage[:, :])
                    else:
                        nc.vector.tensor_copy(w516[:, j, :], wstage[:, :])

            # W1/W2 are needed during L1's POSTs; the rest are deferred into
            # layer 2's AllGather gap to keep L1's DMA window for gathers
            load_weight(0)
            load_weight(1)

            hT = cp.tile([128, NTILE, 4, 128], dt.float16)
            h_out = cp.tile([128, NTILE, DOUT], dt.float16)
            partial = cp.tile([128, NTILE, DHID], dt.float16)
            pp = pps.tile([64, DOUT], dt.float32)

            def gather_bucket(l, h, t, bi):
                """Prep+trigger the SWDGE gather for bucket (h,t)."""
                q = (2 * bi) % 4
                b = h * NTILE + t
                if l == 1:
                    mtile = mp.tile([128, kpmax, DIN], dt.float32, tag="m32")
                    src, esz = xt_in[h], DIN
                elif l == 5:
                    mtile = mp.tile([128, kpmax, DOUT], dt.float16, tag="mB")
                    src, esz = gshB[h], DOUT
                else:
                    mtile = mp.tile([128, kpmax, DHID], dt.float16, tag="mA")
                    src, esz = gshA[h], DHID
                # zero the skipped -1 tail slots: the gather leaves them
                # unwritten, and NaN garbage there would poison the Seg-0
                # matmul columns (0*NaN=NaN)
                kc = mreg[b] // 128
                if kc < kp[b]:
                    nc.vector.memset(
                        mtile[:, kc:kp[b], :].rearrange("p a b -> p (a b)"),
                        0.0)
                # split the bucket across both SWDGE queues so the two
                # transfers overlap (one queue's ring serializes batches)
                ka = (kp[b] + 1) // 2
                for s, (k0, k1) in enumerate(((0, ka), (ka, kp[b]))):
                    if k1 <= k0:
                        continue
                    nreg = min(mreg[b], k1 * 128) - min(mreg[b], k0 * 128)
                    if nreg <= 0:
                        continue
                    nc.gpsimd.dma_gather(
                        out_ap=mtile[:, k0:k1, :],
                        in_ap=src[:, :],
                        idxs_ap=gidx_sb[:, (cbase[b] + k0) * 8:
                                        (cbase[b] + k1) * 8],
                        num_idxs=(k1 - k0) * 128,
                        num_idxs_reg=nreg,
                        elem_size=esz,
                        single_packet=False,
                        queue_num=(q + s) % 4,
                    )
                return mtile

            def agg_bucket(l, h, t, mtile):
                """One-hot matmul accumulation of bucket (h,t) into a fresh
                PSUM aggregator; returns the aggregator tile."""
                b = h * NTILE + t
                fo = DIN if l == 1 else FO[l - 1]
                if l == 1:
                    m16 = mp.tile([128, kpmax, DIN], dt.float16, tag="m16")
                    nc.scalar.activation(
                        m16[:, :kp[b], :].rearrange("p a b -> p (a b)"),
                        mtile[:, :kp[b], :].rearrange("p a b -> p (a b)"),
                        AF.Copy)
                    mtile = m16
                pa = aps.tile([128, DHID], dt.float32, tag="pa")
                for k in range(kp[b]):
                    nc.tensor.matmul(
                        pa[:, :fo], seg_sb[:, cbase[b] + k, :], mtile[:, k, :],
                        start=(k == 0), stop=(k == kp[b] - 1))
                return pa

            def post_tile(l, t):
                """After both halves of tile t are aggregated for layer l:
                finish the tile and stage the next layer's table."""
                fo = DIN if l == 1 else FO[l - 1]
                hsum = wp.tile([128, fo], dt.float32, tag=f"hsum{fo}")
                nc.vector.tensor_tensor(
                    hsum[:, :], post_tile.pa[:, :fo], partial[:, t, :fo],
                    mybir.AluOpType.add)
                if l == 1:
                    # (Ax) -> fp16 -> transpose -> @W1 + b1 -> relu
                    st16 = wp.tile([128, DIN], dt.float16, tag="st16")
                    nc.scalar.activation(st16[:, :], hsum[:, :], AF.Copy)
                    pt1 = tps.tile([128, DHID], dt.float16, tag="pt")
                    nc.tensor.transpose(pt1[:, :128], st16[:, :], id16[:, :])
                    xT = wp.tile([128, DIN], dt.float16, tag="xT")
                    nc.vector.tensor_copy(xT[:, :], pt1[:, :128])
                    pg = gps.tile([128, DHID], dt.float32, tag="pg")
                    nc.tensor.matmul(pg[:, :], xT[:, :], w16[:, 0, :],
                                     start=True, stop=True)
                    hs2 = wp.tile([128, DHID], dt.float32, tag="hsum512")
                    nc.vector.tensor_tensor(
                        hs2[:, :], pg[:, :], breps[:, 0, :],
                        mybir.AluOpType.add)
                    hnm = wp.tile([128, DHID], dt.float16, tag="hnm")
                    nc.scalar.activation(hnm[:, :], hs2[:, :], AF.Relu)
                elif l < 5:
                    hnm = wp.tile([128, fo], dt.float16, tag="hnm")
                    nc.scalar.activation(hnm[:, :], hsum[:, :], AF.Relu)
                else:
                    nc.scalar.activation(h_out[:, t, :], hsum[:, :], AF.Relu)
                    nc.tensor.matmul(
                        pp[:, :], pool_sb[:, t, :64], h_out[:, t, :],
                        start=(t == 0), stop=(t == NTILE - 1))
                    return

                # transposes -> hT -> GEMM W_{l+1} -> cast -> bounce half
                fon = FO[l]
                bounce = bounceA if l < 4 else bounceB
                pt = tps.tile([128, DHID], dt.float16, tag="pt")
                for j in range(4):
                    nc.tensor.transpose(
                        pt[:, j * 128:(j + 1) * 128],
                        hnm[:, j * 128:(j + 1) * 128], id16[:, :])
                nc.vector.tensor_copy(
                    hT[:, t, :, :].rearrange("p a b -> p (a b)"), pt[:, :512])
                pg2 = gps.tile([128, fon], dt.float32, tag="pg")
                for j in range(4):
                    wslot = (w16[:, 1 + 4 * (l - 1) + j, :] if l < 4
                             else w516[:, j, :])
                    nc.tensor.matmul(pg2[:, :], hT[:, t, j, :], wslot,
                                     start=(j == 0), stop=(j == 3))
                hw16 = wp.tile([128, fon], dt.float16, tag="hw16")
                nc.scalar.activation(hw16[:, :], pg2[:, :], AF.Copy)
                hh, r = t // 5, (t % 5) * 128
                nc.sync.dma_start(out=bounce[hh].ap()[r:r + 128, :],
                                  in_=hw16[:, :])

            def ag_half(bounce, gsh):
                nc.gpsimd.collective_compute(
                    "AllGather", mybir.AluOpType.bypass,
                    replica_groups=[list(range(C))],
                    ins=[bounce.ap().opt()],
                    outs=[gsh.ap().opt()])

            # ========================= LAYERS 1..5 =========================
            # Pool-stream order per layer: [AG(l,h0) trigger] h0 gathers,
            # [AG(l,h1) trigger] h1 gathers. The AG triggers wait on the
            # previous layer's bounce writes; placing them at half-pass
            # heads keeps them from head-of-line-blocking gather issue.
            # post_tile lags the aggregation by one bucket so its
            # DVE->ACT->PE chain hides under the next bucket's matmuls.
            bi = 0
            for l in range(1, 6):
                fo = DIN if l == 1 else FO[l - 1]
                # deferred const loads ride this layer's AllGather gap
                # (W_{l+1}/b_{l} are first needed during layer l's passes)
                if l == 2:
                    load_weight(2)
                    nc.sync.dma_start(out=breps[:, 2, :], in_=b_in[2][:, :])
                elif l == 3:
                    load_weight(3)
                    nc.sync.dma_start(out=breps[:, 3, :], in_=b_in[3][:, :])
                elif l == 4:
                    load_weight(4)
                    nc.sync.dma_start(out=brep5[:, :], in_=b_in[4][:, :])
                    nc.sync.dma_start(out=pool_sb[:, :, :], in_=pool_in[:, :, :])
                for h in range(2):
                    pend = None
                    for t in range(NTILE):
                        mt = gather_bucket(l, h, t, bi); bi += 1
                        if l < 5 and h == 1 and t == 8:
                            # trigger the next layer's h0 table AllGather
                            # from this layer's gather tail (POST(l,t0..4)
                            # bounces land around now)
                            ag_half((bounceA if l < 4 else bounceB)[0],
                                    (gshA if l < 4 else gshB)[0])
                        pa = agg_bucket(l, h, t, mt)
                        if h == 0:
                            # fold the post-agg bias into the partial copy
                            if l == 1:
                                nc.vector.tensor_copy(
                                    partial[:, t, :fo], pa[:, :fo])
                            else:
                                nc.vector.tensor_tensor(
                                    partial[:, t, :fo], pa[:, :fo],
                                    breps[:, l - 1, :fo] if l < 5
                                    else brep5[:, :],
                                    mybir.AluOpType.add)
                        else:
                            if pend is not None:
                                post_tile.pa = pend[1]
                                post_tile(l, pend[0])
                            pend = (t, pa)
                    if h == 1:
                        post_tile.pa = pend[1]
                        post_tile(l, pend[0])
                        if l < 5:
                            # h1 table AllGather: emitted after POST(l,t9)
                            # exists; hides under AG(h0) + the h0 gather pass
                            ag_half((bounceA if l < 4 else bounceB)[1],
                                    (gshA if l < 4 else gshB)[1])

            # ---- mean pool: AllReduce over cores ----
            pres = wp.tile([64, DOUT], dt.float32, tag="pres")
            nc.vector.tensor_copy(pres[:, :], pp[:, :])
            nc.sync.dma_start(out=pool_bounce[:, :], in_=pres[:, :])
            nc.gpsimd.collective_compute(
                "AllReduce", mybir.AluOpType.add,
                replica_groups=[list(range(C))],
                ins=[pool_bounce.ap().opt()],
                outs=[pool_sh.ap().opt()])
            ores = wp.tile([64, DOUT], dt.float32, tag="ores")
            nc.sync.dma_start(out=ores[:, :], in_=pool_sh[:, :])
            nc.sync.dma_start(out=out[:, :], in_=ores[:, :])

    nc.compile()
    return nc


_CACHE = {}


def _get_program(meta):
    if meta not in _CACHE:
        _CACHE[meta] = _build(meta)
    return _CACHE[meta]


def make_in_maps(inputs):
    edge_index = np.asarray(inputs["edge_index"])
    batch = np.asarray(inputs["batch"])
    x = np.asarray(inputs["x"], dtype=np.float32)
    gidx_w, seg, pool, meta = _preprocess(edge_index, batch)
    xt0, xt1 = _xtables(x)
    ident = np.eye(128, dtype=np.float16)
    in_maps = []
    for c in range(C):
        m = {
            "xt0_in": xt0,
            "xt1_in": xt1,
            "seg_in": np.ascontiguousarray(seg[c]),
            "gidx_in": gidx_w[c],
            "pool_in": np.ascontiguousarray(pool[c]),
            "id_in": ident,
        }
        for i in range(5):
            w = np.asarray(inputs[f"W{i + 1}"], dtype=np.float32)
            b = np.asarray(inputs[f"b{i + 1}"], dtype=np.float32)
            m[f"w{i}_in"] = w
            m[f"b{i}_in"] = np.ascontiguousarray(np.tile(b[None, :], (128, 1)))
        in_maps.append(m)
    return in_maps, meta


def kernel(**inputs):
    in_maps, meta = make_in_maps(inputs)
    nc = _get_program(meta)
    res = bass_utils.run_bass_kernel_spmd(
        nc, in_maps, core_ids=list(range(C)))
    return res.results[0]["out"].astype(np.float32)
